# revision 1
# baseline (speedup 1.0000x reference)
"""Trainium2 Bass kernel for Autoformer AutoCorrelation attention.

Reference computation (per batch b):
  q = (x_q @ Wq + bq), k, v likewise                               (L, D)
  corr[h,e,:] = irfft(rfft(q_che) * conj(rfft(k_che)))             circular cross-corr
  mean_value[b,:] = corr.mean(over H,E)                            (L,)
  index = top_k(mean_value.mean(over B), 7)                        global over batch
  w = softmax(mean_value[b, index])
  agg = sum_k w[k] * roll(v, -index[k], axis=time)
  out = agg @ Wo + bo

Key identities used (no FFT needed):
  * mean_value[b, tau] = (1/D) * sum_t <q_proj[t+tau], k_proj[t]>  = circular
    diagonal sums of the Gram matrix G = q_proj @ k_proj^T, computed as
    matmuls on the PE with a window trick so that each PSUM column holds a
    fixed (t' - t) difference; a log2(128)-step shifted partition fold then
    yields all 2048 diagonal sums.
  * bq/bk are irrelevant to the stat: with circular sums they add a
    tau-independent constant, and both top-k and softmax are shift-invariant.
  * sum_k w[k] = 1, so bo can be added once to the aggregated output.

Structure (per core, one batch item):
  phase A: q-proj, k-proj (PE, bf16 weights/activations - the stat's top-7
           margin is 12x the bf16 noise), gram (PE, bf16), fold (Pool),
           AllReduce (8KB).
  phase B: v-proj (bf16), O = v_proj @ Wo in natural (t,d) layout via
           lhsT=v^T-slices (no output transposes), O stored bf16 in DRAM.
           Meanwhile: top-k of the global stat, per-batch softmax weights.
  phase C: per time tile: 7 indirect-DMA row gathers of O (one per lag,
           single-offset-column form), then PE accumulation into PSUM with
           lhsT = diag(w_k) (runtime-scaled identity, bf16); the
           host-computed b' = bv @ Wo + bo is added partition-broadcast in
           the PSUM->SBUF copy on the DVE. The PE is mostly idle here; the
           phase is paced by SWDGE descriptor generation on the Pool engine
           (~1.04us fixed cost x 112 gather instructions ~= 116us).

Hardware constraints found by experiment (do not regress):
  - Multi-column indirect-DMA offset APs ([128, >1]) gather wrong/corrupt
    rows (verified with a ramp-pattern jig); only [128, 1] offset columns
    are reliable, which forces the per-tile gather count above.
  - Future lead for the tail: each (lag, tile) gather is a CONTIGUOUS
    128-row block of a doubled-O layout, so a register-offset regular DMA
    could replace SWDGE gathers and leave the Pool queue. The mechanism
    exists: lea(reg, ap) + RegisterAccessPattern lowering, exposed via
    remote_dma/remote_dma_fused's SRC_DST_REG variant (bass.py ~4740) and
    the PSEUDO_DMA_DIRECT2D register path (bass.py ~2185). Open question:
    loopback remote_dma to self, or a local dynamic-DGE wrapper. Upside is
    bounded though: the tail's DMA engines are already 80% busy (106us
    floor vs the 116us Pool pacing), so max win is ~10-25us.
  - Mixing f32/f32r with 16-bit matmul inputs is rejected; f32r inputs must
    come from f32r-typed DMAs or activation casts (not memset/bitcast).
  - f32r and bf16 memsets are not ISA-encodable; GPSIMD cannot touch PSUM;
    casts of small/strided tiles must go through activation, not DMA.

All matmuls 1 cycle/row; projections/gram/agg are bf16 x bf16, transposes
f32r x f32r identity. Projections are software-pipelined ACROSS projections:
the next chunk's transposes are emitted before the current chunk's matmuls.
"""

import functools
import os
import sys

sys.path.insert(0, "/opt/trn_rl_repo")

import numpy as np

import concourse.bass as bass
import concourse.mybir as mybir
import concourse.tile as tile
from concourse import bacc, bass_utils
from concourse.masks import make_identity

P = 128
B, L, D = 8, 2048, 1024
NCORES = 8
TOPK = 7
KC = D // P   # 8 contraction chunks
MT = D // P   # 8 output-row tiles
TT = L // P   # 16 time tiles
TC = 512      # time-chunk width for projections
NTCH = L // TC
GW = 512      # gram matmul moving width (one PSUM bank)
QT = 4        # time tiles per phase-C quarter pass

f32 = mybir.dt.float32
f32r = mybir.dt.float32r
bf16 = mybir.dt.bfloat16
u32 = mybir.dt.uint32
i32 = mybir.dt.int32
AF = mybir.ActivationFunctionType
ALU = mybir.AluOpType

DEBUG_OUTS = os.environ.get("KERNEL_DEBUG", "0") == "1"
SIM_MODE = os.environ.get("KERNEL_SIM", "0") == "1"   # single-core, no collective


def _build():
    nc = bacc.Bacc("TRN2", target_bir_lowering=False, debug=False,
                   num_devices=1 if SIM_MODE else NCORES)

    x_q = nc.dram_tensor("queries", [L, D], f32r, kind="ExternalInput").ap()
    x_k = nc.dram_tensor("keys", [L, D], f32r, kind="ExternalInput").ap()
    x_v = nc.dram_tensor("values", [L, D], f32r, kind="ExternalInput").ap()
    w_q = nc.dram_tensor("Wq", [D, D], f32r, kind="ExternalInput").ap()
    w_k = nc.dram_tensor("Wk", [D, D], f32r, kind="ExternalInput").ap()
    w_v = nc.dram_tensor("Wv", [D, D], f32r, kind="ExternalInput").ap()
    w_o = nc.dram_tensor("Wo", [D, D], f32r, kind="ExternalInput").ap()
    b_q = nc.dram_tensor("bq", [D], f32, kind="ExternalInput").ap()
    b_k = nc.dram_tensor("bk", [D], f32, kind="ExternalInput").ap()
    b_v = nc.dram_tensor("bv", [D], f32, kind="ExternalInput").ap()
    b_o = nc.dram_tensor("bo", [D], f32, kind="ExternalInput").ap()
    b_p = nc.dram_tensor("bprime", [D], f32, kind="ExternalInput").ap()
    out = nc.dram_tensor("out", [L, D], f32, kind="ExternalOutput").ap()
    if DEBUG_OUTS:
        dbg_mv = nc.dram_tensor("dbg_mv", [1, L], f32, kind="ExternalOutput").ap()
        dbg_mvsum = nc.dram_tensor("dbg_mvsum", [1, L], f32, kind="ExternalOutput").ap()
        dbg_idx = nc.dram_tensor("dbg_idx", [1, 8], u32, kind="ExternalOutput").ap()
        dbg_w = nc.dram_tensor("dbg_w", [1, 8], f32, kind="ExternalOutput").ap()

    from contextlib import ExitStack

    with tile.TileContext(nc) as tc:
        with (
            tc.tile_pool(name="smalls", bufs=1) as smalls,
            tc.tile_pool(name="dram", bufs=1, space="DRAM") as dram,
        ):
            proj_es = ExitStack()
            natp = proj_es.enter_context(tc.tile_pool(name="natp", bufs=4, side="right"))
            xtp = proj_es.enter_context(tc.tile_pool(name="xtp", bufs=1, side="right"))
            wp = proj_es.enter_context(tc.tile_pool(name="wp", bufs=1, side="right"))
            psT = proj_es.enter_context(tc.tile_pool(name="psT", bufs=2, space="PSUM"))
            psP = proj_es.enter_context(tc.tile_pool(name="psP", bufs=2, space="PSUM"))
            # two explicit staging buffers (ping-pong across chunks)
            xtp_tiles = [xtp.tile([P, KC, TC], bf16, tag=f"xt{i}", name=f"xt{i}")
                         for i in range(2)]

            # x_v^T is staged persistently (all 2048 columns): the O'
            # projection reads it in two passes (low/high d-half), and it is
            # filled starting before the gram. vtp/otp stay open through
            # phase C so the phase-C pools never land on their bytes (the
            # opening barrier would serialize gathers behind O-hi).
            vtp_es = ExitStack()
            vtp = vtp_es.enter_context(tc.tile_pool(name="vtp", bufs=1))
            otp = vtp_es.enter_context(tc.tile_pool(name="otp", bufs=3))
            xv_t = vtp.tile([P, KC, L], bf16, tag="xv_t", name="xv_t")

            # f32r memsets are not ISA-encodable: build the identity in f32,
            # then activation-cast to f32r for the transposes.
            ident_f = smalls.tile([P, P], f32, tag="ident_f")
            make_identity(nc, ident_f[:])
            ident_r = smalls.tile([P, P], f32r, tag="ident_r")
            nc.scalar.activation(ident_r[:], ident_f[:], AF.Identity)
            ident = ident_r[:]

            # Weights cast-loaded f32 -> bf16 by the gpsimd (SWDGE) DMA.
            # wq in two half loads (emitted after q's first x slabs win the
            # DMA queue); the others are emitted lazily below.
            w_sb = {n: wp.tile([P, KC, D], bf16, tag=f"w_{n}", name=f"w_{n}")
                    for n in "qko"}

            def load_w(name, wdram, halves=1):
                src = wdram.rearrange("(kc p) d -> p kc d", p=P)
                hw = D // halves
                for hh in range(halves):
                    nc.gpsimd.dma_start(w_sb[name][:, :, hh * hw:(hh + 1) * hw],
                                        src[:, :, hh * hw:(hh + 1) * hw])

            # b' = bv @ Wo + bo is computed on the host (tiny matvec) and
            # applied in phase C as a partition-broadcast DVE add (sum_k w_k
            # = 1, so the bias is added once after the lag aggregation).
            b_bc = smalls.tile([P, D], f32, tag="b_bc")

            # static index helpers for the gather (no data dependency)
            c3968 = smalls.tile([1, 8], i32, tag="c3968")
            iota2 = smalls.tile([P, TT], i32, tag="iota2")
            c2047 = smalls.tile([P, 1], i32, tag="c2047")

            def emit_smalls_setup():
                # bp_row borrows the fold pool's B buffer (dead until the
                # post-gram fold; the broadcast below is its only reader)
                bp_row = foldp.tile([1, D], f32, tag="foldB")
                nc.sync.dma_start(bp_row[:], b_p.rearrange("d -> () d"))
                nc.gpsimd.partition_broadcast(b_bc[:], bp_row[:])
                nc.gpsimd.memset(c3968[:], 2 * L)
                nc.gpsimd.iota(iota2[:], pattern=[[P, TT]], base=0,
                               channel_multiplier=1)
                nc.gpsimd.memset(c2047[:], L - 1)

            # ---- software-pipelined projection helpers ----------------------
            def emit_transposes(x_dram, seq, dst=None):
                """Stage time-chunk seq of x^T into xtp_tiles[seq % 2], or
                into the persistent [P, KC, L] tile ``dst``."""
                c = seq % NTCH
                if dst is None:
                    xt, col0 = xtp_tiles[seq % 2], 0
                else:
                    xt, col0 = dst, c * TC
                for j in range(TC // P):
                    nat = natp.tile([P, D], f32r, tag="nat")
                    eng = nc.sync if j % 2 == 0 else nc.scalar
                    eng.dma_start(nat[:],
                                  x_dram[(c * TC + j * P):(c * TC + (j + 1) * P), :])
                    for h in range(2):
                        pst = psT.tile([P, 4 * P], f32r, tag="psT")
                        for q in range(4):
                            kc = h * 4 + q
                            nc.tensor.transpose(pst[:, q * P:(q + 1) * P],
                                                nat[:, kc * P:(kc + 1) * P], ident)
                        nc.vector.tensor_copy(
                            xt[:, h * 4:(h + 1) * 4, col0 + j * P:col0 + (j + 1) * P],
                            pst[:].rearrange("p (a b) -> p a b", a=4))

            def emit_wT(w_dram, dst):
                """Stage W^T (bf16) into ``dst`` [P, KC, D]: dst[p, kc, i] =
                W[i, kc*128 + p]. 8 row-slabs, transposed on the PE."""
                for sl in range(KC):
                    nat = natp.tile([P, D], f32r, tag="nat")
                    eng = nc.sync if sl % 2 == 0 else nc.scalar
                    eng.dma_start(nat[:], w_dram[sl * P:(sl + 1) * P, :])
                    for h in range(2):
                        pst = psT.tile([P, 4 * P], f32r, tag="psT")
                        for q in range(4):
                            kc = h * 4 + q
                            nc.tensor.transpose(pst[:, q * P:(q + 1) * P],
                                                nat[:, kc * P:(kc + 1) * P], ident)
                        nc.vector.tensor_copy(
                            dst[:, h * 4:(h + 1) * 4, sl * P:(sl + 1) * P],
                            pst[:].rearrange("p (a b) -> p a b", a=4))

            def emit_matmuls(wname, bias_col, out_tiles, seq, post=None):
                xt = xtp_tiles[seq % 2]
                c = seq % NTCH
                for m in range(MT):
                    psp = psP.tile([P, TC], f32, tag="psP")
                    for kc in range(KC):
                        nc.tensor.matmul(psp[:], w_sb[wname][:, kc, m * P:(m + 1) * P],
                                         xt[:, kc, :], start=(kc == 0),
                                         stop=(kc == KC - 1))
                    if bias_col is None:
                        nc.scalar.activation(out_tiles[m][:, c * TC:(c + 1) * TC],
                                             psp[:], AF.Identity)
                    else:
                        nc.scalar.activation(out_tiles[m][:, c * TC:(c + 1) * TC],
                                             psp[:], AF.Identity,
                                             bias=bias_col[:, m:m + 1])
                if post is not None:
                    post(c)

            # ---------------- phase A: q/k projections + gram + fold ----------
            foldp = proj_es.enter_context(tc.tile_pool(name="foldp", bufs=1))
            with (
                tc.tile_pool(name="qkp", bufs=1) as qkp,
                tc.tile_pool(name="psG", bufs=1, space="PSUM") as psG,
            ):
                q_t = [qkp.tile([P, L], bf16, tag=f"qT{m}", name=f"qT{m}") for m in range(MT)]
                k_t = [qkp.tile([P, L], bf16, tag=f"kT{m}", name=f"kT{m}") for m in range(MT)]

                # global pipelined schedule over q (seq 0-3) and k (seq 4-7).
                # Startup: T0 M0 T1 T2 M1 T3 M2 ... - M(q0) is not queued
                # behind a transpose that waits on later DMA, and the
                # one-chunk lookahead (T(s+1) before M(s)) builds up during
                # M(q0). Weight loads are placed so early x slabs win the
                # DMA queue.
                def xof(s):
                    return x_q if s <= 3 else x_k if s <= 7 else x_v

                emit_transposes(x_q, 0)
                load_w("q", w_q, halves=2)
                emit_matmuls("q", None, q_t, 0)
                emit_transposes(x_q, 1)
                for s in range(1, 8):
                    if s + 1 <= 7:
                        emit_transposes(xof(s + 1), s + 1)
                    elif s + 1 == 8:
                        # v chunk 0 staged (persistent) before the gram so
                        # the O' matmuls can start the moment Wvo is ready.
                        emit_transposes(x_v, 8, dst=xv_t)
                    emit_matmuls("q" if s <= 3 else "k", None,
                                 q_t if s <= 3 else k_t, s)
                    if s == 1:
                        load_w("k", w_k)
                    elif s == 2:
                        load_w("o", w_o)
                        emit_smalls_setup()

                # Gram: psg[r, q] accumulates G[128i + r, (128i + q) % L]
                # over i, mc  =>  tau = (r - q) mod L per element. Window
                # start 128i so that i == 0 (the start=True pass) is exactly
                # bank-aligned.
                psg = psG.tile([P, L], f32, tag="psG")
                for i in range(TT):
                    for mc in range(MT):
                        lhs = q_t[mc][:, i * P:(i + 1) * P]
                        st = (i == 0 and mc == 0)
                        sp = (i == TT - 1 and mc == MT - 1)
                        for g in range(L // GW):
                            s = (P * i + GW * g) % L
                            e = s + GW
                            if e <= L:
                                nc.tensor.matmul(psg[:, g * GW:(g + 1) * GW], lhs,
                                                 k_t[mc][:, s:e], start=st, stop=sp,
                                                 skip_group_check=True)
                            else:
                                a = L - s
                                nc.tensor.matmul(psg[:, g * GW:g * GW + a], lhs,
                                                 k_t[mc][:, s:L], start=st, stop=sp,
                                                 skip_group_check=True)
                                nc.tensor.matmul(psg[:, g * GW + a:(g + 1) * GW], lhs,
                                                 k_t[mc][:, 0:e - L], start=st, stop=sp,
                                                 skip_group_check=True)

                # fold: S[q] = sum_r psg[r, (q + r) % L]; 7 shifted halvings.
                # mv_own[q] = sum over dout of corr[b, tau], tau = (-q) % L.
                # Runs entirely on the Pool queue (adds + SBUF shifts) so the
                # DVE/Act/sync queues stay clear for the v/O projections.
                # step 1 reads the gram PSUM - GPSIMD cannot access PSUM, so
                # these three ops go on DVE (v's first-chunk copies are
                # emitted earlier, so this costs nothing structurally).
                tmp64 = foldp.tile([64, L], f32, tag="foldA")
                nc.vector.tensor_copy(tmp64[:], psg[64:128, :])
                f0 = foldp.tile([64, L], f32, tag="foldB")
                nc.vector.tensor_add(f0[:, 0:L - 64], psg[0:64, 0:L - 64],
                                     tmp64[:, 64:L])
                nc.vector.tensor_add(f0[:, L - 64:L], psg[0:64, L - 64:L],
                                     tmp64[:, 0:64])
                src = f0
                tag_flip = True
                for h in (32, 16, 8, 4, 2, 1):
                    tmp = foldp.tile([h, L], f32, tag="foldA" if tag_flip else "foldB")
                    tag_flip = not tag_flip
                    nc.gpsimd.dma_start(tmp[:, 0:L - h], src[h:2 * h, h:L])
                    nc.gpsimd.dma_start(tmp[:, L - h:L], src[h:2 * h, 0:h])
                    nc.gpsimd.tensor_add(tmp[:], src[0:h, :], tmp[:])
                    src = tmp
                mv_own = src  # (1, L) tile

                # ---------------- collective: sum over batch ------------------
                ar_in = dram.tile([1, L], f32, tag="ar_in")
                ar_out = dram.tile([1, L], f32, tag="ar_out")
                nc.gpsimd.dma_start(ar_in[:], mv_own[:])
                if DEBUG_OUTS:
                    nc.sync.dma_start(dbg_mv, mv_own[:])

            if SIM_MODE:
                nc.gpsimd.dma_start(ar_out[:], ar_in[:])
            else:
                nc.gpsimd.collective_compute(
                    "AllReduce", ALU.add,
                    replica_groups=[list(range(NCORES))],
                    ins=[ar_in[:].opt()], outs=[ar_out[:].opt()],
                )
            O_dram = dram.tile([L, D], bf16, tag="O_dram")

            # ---------------- phase B: v proj + O = v_proj @ Wo (t,d) --------
            with (
                tc.tile_pool(name="psF", bufs=2, space="PSUM") as psF,
            ):
                # Wvo = Wv @ Wo (bf16): stage Wv^T (PE transposes; the slab
                # DMAs ran during the gram), then 128 accumulating matmuls.
                # Reuses the dead w_q / w_k SBUF allocations.
                wvT = wp.tile([P, KC, D], bf16, tag="w_q")
                emit_wT(w_v, wvT[:])
                wvo = wp.tile([P, KC, D], bf16, tag="w_k")
                for it in range(MT):
                    for half in range(2):
                        pso = psF.tile([P, GW], f32, tag="psF")
                        for mc in range(KC):
                            nc.tensor.matmul(
                                pso[:],
                                wvT[:, mc, it * P:(it + 1) * P],
                                w_sb["o"][:, mc, half * GW:(half + 1) * GW],
                                start=(mc == 0), stop=(mc == KC - 1))
                        nc.scalar.activation(wvo[:, it, half * GW:(half + 1) * GW],
                                             pso[:], AF.Identity)


                # O' = x_v @ Wvo in natural (t,d) layout, lhsT = xv_t slices.
                # Low half per chunk (as soon as its xv_t columns land), high
                # half in a second pass that overlaps the low-half gathers.
                def emit_o_tile(i):
                    osb = otp.tile([P, D], bf16, tag="osb")
                    for half in range(2):
                        pso = psF.tile([P, GW], f32, tag="psF")
                        for dc in range(KC):
                            nc.tensor.matmul(
                                pso[:],
                                xv_t[:, dc, i * P:(i + 1) * P],
                                wvo[:, dc, half * GW:(half + 1) * GW],
                                start=(dc == 0), stop=(dc == KC - 1))
                        nc.scalar.activation(osb[:, half * GW:(half + 1) * GW],
                                             pso[:], AF.Identity)
                    (nc.sync if i % 2 == 0 else nc.scalar).dma_start(
                        O_dram[i * P:(i + 1) * P, :], osb[:])

                for c in range(NTCH):
                    if c + 1 < NTCH:
                        emit_transposes(x_v, 8 + c + 1, dst=xv_t)
                    for i in range(4 * c, 4 * c + 4):
                        emit_o_tile(i)

                # ---- top-k over the batch-summed stat (emitted after all
                # O-proj work so the in-order queues never stall O) ----------
                mv_sum = foldp.tile([1, L], f32, tag="foldA")
                nc.gpsimd.dma_start(mv_sum[:], ar_out[:])
                top_vals = smalls.tile([1, 8], f32, tag="top_vals")
                top_idx = smalls.tile([1, 8], u32, tag="top_idx")
                nc.vector.max_with_indices(top_vals[:], top_idx[:], mv_sum[:])

                # own-batch values at the top-k positions (for the softmax):
                # SBUF free<->partition transposes bounce via DRAM.
                idx_bounce = dram.tile([1, 8], u32, tag="idx_bounce")
                nc.gpsimd.dma_start(idx_bounce[:], top_idx[:])
                idx_col = smalls.tile([8, 1], u32, tag="idx_col")
                nc.gpsimd.dma_start(idx_col[:],
                                    idx_bounce[:].rearrange("o k -> k o"))
                wvals_col = smalls.tile([8, 1], f32, tag="wvals_col")
                nc.gpsimd.indirect_dma_start(
                    out=wvals_col[:], out_offset=None,
                    in_=ar_in[:].rearrange("o q -> (o q) ()"),
                    in_offset=bass.IndirectOffsetOnAxis(ap=idx_col[:, 0:1],
                                                        axis=0),
                )
                wv_bounce = dram.tile([1, 8], f32, tag="wv_bounce")
                nc.gpsimd.dma_start(wv_bounce[:].rearrange("o k -> k o"),
                                    wvals_col[:])
                wvals = smalls.tile([1, 8], f32, tag="wvals")
                nc.gpsimd.dma_start(wvals[:], wv_bounce[:])

                # lag for mv position q is tau = (-q) % L; gather row
                # indices per time-tile: idx[p, i] = (tau + 128*i + p) % L.
                taus_row = smalls.tile([1, 8], i32, tag="taus_row")
                nc.vector.tensor_tensor(taus_row[:], c3968[:],
                                        top_idx[:].bitcast(i32),
                                        ALU.subtract)
                taus_bc = smalls.tile([P, 8], i32, tag="taus_bc")
                nc.gpsimd.partition_broadcast(taus_bc[:], taus_row[:])
                idx_k = []
                for k2 in range(TOPK):
                    ik = smalls.tile([P, TT], i32, tag=f"idx_k{k2}",
                                     name=f"idx_k{k2}")
                    nc.vector.tensor_tensor(
                        ik[:], taus_bc[:, k2:k2 + 1].to_broadcast((P, TT)),
                        iota2[:], ALU.add)
                    nc.vector.tensor_tensor(
                        ik[:], ik[:], c2047[:].to_broadcast((P, TT)),
                        ALU.bitwise_and)
                    idx_k.append(ik)

                # ---- per-batch softmax weights ---------------------------
                w7 = smalls.tile([P, 8], f32, tag="w7")
                nc.gpsimd.partition_broadcast(w7[:], wvals[:])
                wmax = smalls.tile([P, 1], f32, tag="wmax")
                nc.vector.tensor_reduce(wmax[:], w7[:, 0:TOPK],
                                        mybir.AxisListType.X, ALU.max)
                negmax = smalls.tile([P, 1], f32, tag="negmax")
                nc.vector.tensor_scalar_mul(negmax[:], wmax[:], -1.0 / D)
                wexp = smalls.tile([P, 8], f32, tag="wexp")
                nc.scalar.activation(wexp[:, 0:TOPK], w7[:, 0:TOPK], AF.Exp,
                                     bias=negmax[:], scale=1.0 / D)
                wsum = smalls.tile([P, 1], f32, tag="wsum")
                nc.vector.tensor_reduce(wsum[:], wexp[:, 0:TOPK],
                                        mybir.AxisListType.X, ALU.add)
                wrec = smalls.tile([P, 1], f32, tag="wrec")
                nc.vector.reciprocal(wrec[:], wsum[:])
                wfin = smalls.tile([P, 8], f32, tag="wfin")
                nc.vector.tensor_scalar_mul(wfin[:, 0:TOPK], wexp[:, 0:TOPK],
                                            wrec[:])

                # scaled identities diag(w_k) for the phase-C accumulation
                # (bf16: the verifier requires 16-bit lhsT with bf16 rhs)
                wdiag = []
                for k2 in range(TOPK):
                    wd = smalls.tile([P, P], bf16, tag=f"wdiag{k2}",
                                     name=f"wdiag{k2}")
                    nc.vector.tensor_scalar_mul(wd[:], ident_f[:],
                                                wfin[:, k2:k2 + 1])
                    wdiag.append(wd)

                if DEBUG_OUTS:
                    nc.sync.dma_start(dbg_mvsum, mv_sum[:])
                    nc.sync.dma_start(dbg_idx, top_idx[:])
                    nc.sync.dma_start(dbg_w, wfin[:1, :])

            proj_es.close()

            # ---------------- phase C: gather + PE agg + store ---------------
            # Per d-half, per quarter (4 time tiles): 7 gathers [128, 4, 512]
            # (one per lag), then per time-tile PSUM group: 7 diag(w_k)
            # matmuls + one ones-row matmul adding bo's half, accumulated in
            # f32 PSUM. PSUM tiles ping-pong so one quarter's copy/store
            # overlaps the next quarter's matmuls.
            with (
                tc.tile_pool(name="gthp", bufs=2) as gthp,
                tc.tile_pool(name="psU", bufs=2, space="PSUM") as psU,
                tc.tile_pool(name="outp", bufs=2) as outp,
            ):
                out_r = out.rearrange("(i p) d -> p i d", p=P)
                for i in range(TT):
                    gths = []
                    for k2 in range(TOPK):
                        gth = gthp.tile([P, D], bf16, tag=f"gth{k2}",
                                        name=f"gth{k2}_{i % 2}")
                        nc.gpsimd.indirect_dma_start(
                            out=gth[:], out_offset=None,
                            in_=O_dram[:],
                            in_offset=bass.IndirectOffsetOnAxis(
                                ap=idx_k[k2][:, i:i + 1], axis=0),
                        )
                        gths.append(gth)
                    psu = psU.tile([P, D], f32, tag="psU")
                    for half in range(2):
                        win = psu[:, half * GW:(half + 1) * GW]
                        for k2 in range(TOPK):
                            nc.tensor.matmul(
                                win, wdiag[k2][:],
                                gths[k2][:, half * GW:(half + 1) * GW],
                                start=(k2 == 0), stop=(k2 == TOPK - 1),
                                skip_group_check=True)
                    uf = outp.tile([P, D], f32, tag="uf")
                    for half in range(2):
                        nc.vector.tensor_add(
                            uf[:, half * GW:(half + 1) * GW],
                            psu[:, half * GW:(half + 1) * GW],
                            b_bc[:, half * GW:(half + 1) * GW])
                    (nc.sync if i % 2 == 0 else nc.scalar).dma_start(
                        out_r[:, i, :], uf[:])

            vtp_es.close()

    nc.compile()
    return nc


@functools.lru_cache(maxsize=1)
def _get_nc():
    return _build()


def kernel(queries, keys, values, Wq, bq, Wk, bk, Wv, bv, Wo, bo):
    nc = _get_nc()
    shared = {
        "Wq": np.ascontiguousarray(np.asarray(Wq, dtype=np.float32)),
        "Wk": np.ascontiguousarray(np.asarray(Wk, dtype=np.float32)),
        "Wv": np.ascontiguousarray(np.asarray(Wv, dtype=np.float32)),
        "Wo": np.ascontiguousarray(np.asarray(Wo, dtype=np.float32)),
        "bq": np.ascontiguousarray(np.asarray(bq, dtype=np.float32)),
        "bk": np.ascontiguousarray(np.asarray(bk, dtype=np.float32)),
        "bv": np.ascontiguousarray(np.asarray(bv, dtype=np.float32)),
        "bo": np.ascontiguousarray(np.asarray(bo, dtype=np.float32)),
        "bprime": np.ascontiguousarray(
            (np.asarray(bv, np.float64) @ np.asarray(Wo, np.float64)
             + np.asarray(bo, np.float64)).astype(np.float32)),
    }
    queries = np.asarray(queries, dtype=np.float32)
    keys = np.asarray(keys, dtype=np.float32)
    values = np.asarray(values, dtype=np.float32)
    in_maps = []
    for c in range(NCORES):
        m = dict(shared)
        m["queries"] = np.ascontiguousarray(queries[c])
        m["keys"] = np.ascontiguousarray(keys[c])
        m["values"] = np.ascontiguousarray(values[c])
        in_maps.append(m)
    trace = os.environ.get("KERNEL_TRACE", "0") == "1"
    res = bass_utils.run_bass_kernel_spmd(nc, in_maps, core_ids=list(range(NCORES)),
                                          trace=trace)
    kernel.last_results = res
    return np.stack([res.results[c]["out"] for c in range(NCORES)])

